# revision 8
# baseline (speedup 1.0000x reference)
"""Trainium2 Bass kernel for nn_DiagTripleRCell.

Math (per (b, d) element, T steps):
    xv = W_x x + b ; xd = W_delta x + b_delta ; xg = W_gate x + b_gate
    h_{t+1} = (1-delta_t) h_t + delta_t tanh(xv_t + r_h h_t),
      delta_t = sigmoid(xd_t + r_delta h_t)
    out_t = h_{t+1} * silu(xg_t)

Strategy: batch-parallel over 8 cores (B=16 -> 2 per core). GEMMs run on
the PE in f32r (tf32) with outputs laid [d_out partitions, time free] so
the recurrence can use the DVE hardware affine scan (tensor_tensor_scan).
The nonlinear recurrence is solved parallel-in-time by Newton iteration:
an h-independent affine-scan initial guess, then NITER Newton passes,
each of which linearizes around the current trajectory and solves the
resulting affine recurrence exactly with one scan per (d-chunk, b) pair.
Iteration 1 runs in fp16 (accuracy only needs to reach ~3e-3 there),
the final iteration in fp32.
"""
import sys

sys.path.insert(0, "/opt/trn_rl_repo")

import numpy as np

import concourse.bass as bass
import concourse.tile as tile
from concourse import mybir
from concourse.bass_utils import run_bass_kernel_spmd
from concourse.vector_clock import ScopedClock

F32 = mybir.dt.float32
F32R = mybir.dt.float32r
F16 = mybir.dt.float16
AF = mybir.ActivationFunctionType
ALU = mybir.AluOpType

T, B, D = 2048, 16, 1024
NCORES = 8
BL = B // NCORES          # batches per core
EC = D // 128             # output-d chunks
KC = D // 128             # contraction chunks
TC = T // 512             # psum column chunks
NITER = 2                 # Newton iterations after the init scan

# ---------------------------------------------------------------------------
# walrus workaround: this container's walrus accepts at most ONE sem wait per
# instruction; split extras onto single-wait NOPs.
_uid = [0]


def _nop_like(inst, wait):
    _uid[0] += 1
    return mybir.InstNoOp(
        name=f"waitnop_{_uid[0]}", ins=[], outs=[], engine=inst.engine,
        sync_info=mybir.SyncInfo(on_wait=[wait], on_update=[]),
    )


def _split_multi_waits(nc):
    for fn in nc.m.functions:
        for blk in fn.blocks:
            new_insts = []
            for inst in blk.instructions:
                si = getattr(inst, "sync_info", None)
                ow = list(si.on_wait) if (si is not None and si.on_wait) else []
                if len(ow) > 1:
                    for w in ow[:-1]:
                        new_insts.append(_nop_like(inst, w))
                    inst.sync_info = mybir.SyncInfo(
                        on_wait=[ow[-1]], on_update=list(si.on_update or []))
                new_insts.append(inst)
            blk.instructions = new_insts


def _patched_drain_and_barrier(self, tick_clock, wait_clock):
    nc = self.nc
    drain_inst = nc.sync.drain()
    wait_clock.add_sem_waits(
        drain_inst.ins, ScopedClock({None: tick_clock.global_clock}))
    si = drain_inst.ins.sync_info
    if si is not None and si.on_wait is not None and len(si.on_wait) > 1:
        waits = list(si.on_wait)
        drain_inst.ins.sync_info = mybir.SyncInfo(
            on_wait=waits[:1], on_update=list(si.on_update or []))
        for w in waits[1:]:
            nop = nc.sync.nop()
            nop.ins.sync_info = mybir.SyncInfo(on_wait=[w], on_update=[])
    nc.all_engine_barrier()
    assert self.sems is not None
    popped = nc._tile_sem_poison_stack.pop()
    assert popped is self._sem_poison
    nc.clear_and_free_semaphores(list(self.sems.allocated().values()))
    nc.all_engine_barrier()


tile.TileContext._drain_and_barrier = _patched_drain_and_barrier

# ---------------------------------------------------------------------------


def _tf32_rne(a):
    """Round fp32 array to tf32 (13 low mantissa bits cleared), RNE."""
    u = np.ascontiguousarray(a, dtype=np.float32).view(np.uint32)
    lsb = (u >> np.uint32(13)) & np.uint32(1)
    r = (u + np.uint32(0xFFF) + lsb) & np.uint32(0xFFFFE000)
    return r.view(np.float32)


def _build_program():
    nc = bass.Bass(trn_type="TRN2", target_bir_lowering=False, debug=False,
                   num_devices=NCORES)

    xt = nc.dram_tensor("xt", [BL, D, T], F32R, kind="ExternalInput").ap()
    wts = [nc.dram_tensor(f"wt{w}", [D, D], F32R, kind="ExternalInput").ap()
           for w in range(3)]  # W^T for x, delta, gate: [d_in, d_out]
    bias = nc.dram_tensor("bias", [3, D, 1], F32, kind="ExternalInput").ap()
    rh_d = nc.dram_tensor("rh", [D, 1], F32, kind="ExternalInput").ap()
    rd_d = nc.dram_tensor("rd", [D, 1], F32, kind="ExternalInput").ap()
    omrh_d = nc.dram_tensor("omrh", [D, 1], F32, kind="ExternalInput").ap()  # 1 - r_h
    h0_d = nc.dram_tensor("h0", [BL, D, 1], F32, kind="ExternalInput").ap()

    out_d = nc.dram_tensor("out_dev", [BL, D, T], F32, kind="ExternalOutput").ap()
    h_out = nc.dram_tensor("h_dev", [BL, D, T + 1], F32, kind="ExternalOutput").ap()

    with tile.TileContext(nc) as tc:
        with tc.tile_pool(name="xb", bufs=1) as xbp, \
             tc.tile_pool(name="wp", bufs=1) as wp, \
             tc.tile_pool(name="cst", bufs=1) as cst, \
             tc.tile_pool(name="psum", bufs=2, space="PSUM") as pp, \
             tc.tile_pool(name="work", bufs=2) as wk:

            rh_t = cst.tile([128, EC], F32, name="rh", tag="rh")
            rd_t = cst.tile([128, EC], F32, name="rd", tag="rd")
            omrh_t = cst.tile([128, EC], F32, name="omrh", tag="omrh")
            bias_t = [cst.tile([128, EC], F32, name=f"bias{w}", tag=f"bias{w}")
                      for w in range(3)]
            for e in range(EC):
                sl = slice(e * 128, (e + 1) * 128)
                nc.sync.dma_start(rh_t[:, e:e + 1], rh_d[sl, :])
                nc.sync.dma_start(rd_t[:, e:e + 1], rd_d[sl, :])
                nc.sync.dma_start(omrh_t[:, e:e + 1], omrh_d[sl, :])
                for w in range(3):
                    nc.sync.dma_start(bias_t[w][:, e:e + 1], bias[w, sl, :])

            xb_cur = [None]

            def emit_G(p):
                b, e = divmod(p, EC)
                esl = slice(e * 128, (e + 1) * 128)
                if e == 0:
                    xb = []
                    for k in range(KC):
                        t = xbp.tile([128, T], F32R, name=f"xb{k}", tag=f"xb{k}")
                        nc.sync.dma_start(t[:], xt[b, k * 128:(k + 1) * 128, :])
                        xb.append(t)
                    xb_cur[0] = xb
                xb = xb_cur[0]

                wt_e = []
                for w in range(3):
                    wcol = []
                    for k in range(KC):
                        wt = wp.tile([128, 128], F32R,
                                     name=f"w{w}_{k}", tag=f"w{w}_{k}")
                        nc.sync.dma_start(
                            wt[:], wts[w][k * 128:(k + 1) * 128, esl])
                        wcol.append(wt)
                    wt_e.append(wcol)

                XV = wk.tile([128, T], F32, name="XV", tag="XV")
                XD = wk.tile([128, T], F32, name="XD", tag="XD")
                XG = wk.tile([128, T], F32, name="XG", tag="XG", bufs=1)
                g32 = wk.tile([128, T + 1], F32, name="g32", tag="g32", bufs=1)
                a0 = wk.tile([128, T], F16, name="a0", tag="a0", bufs=1)
                b0 = wk.tile([128, T], F16, name="b0", tag="b0", bufs=1)

                for t4 in range(TC):
                    tsl = slice(t4 * 512, (t4 + 1) * 512)
                    bv = bias_t[0][:, e:e + 1]
                    bd = bias_t[1][:, e:e + 1]
                    bg = bias_t[2][:, e:e + 1]

                    pv = pp.tile([128, 512], F32, name="pv", tag="pv")
                    for k in range(KC):
                        nc.tensor.matmul(pv[:], wt_e[0][k][:], xb[k][:, tsl],
                                         start=(k == 0), stop=(k == KC - 1))
                    nc.scalar.activation(XV[:, tsl], pv[:], AF.Identity, bias=bv)
                    U0s = wk.tile([128, 512], F16, name="U0s", tag="U0s")
                    nc.scalar.activation(U0s[:], pv[:], AF.Tanh, bias=bv)

                    pd = pp.tile([128, 512], F32, name="pd", tag="pd")
                    for k in range(KC):
                        nc.tensor.matmul(pd[:], wt_e[1][k][:], xb[k][:, tsl],
                                         start=(k == 0), stop=(k == KC - 1))
                    nc.scalar.activation(XD[:, tsl], pd[:], AF.Identity, bias=bd)
                    S0s = wk.tile([128, 512], F16, name="S0s", tag="S0s")
                    nc.scalar.activation(S0s[:], pd[:], AF.Sigmoid, bias=bd)

                    pg = pp.tile([128, 512], F32, name="pg", tag="pg")
                    for k in range(KC):
                        nc.tensor.matmul(pg[:], wt_e[2][k][:], xb[k][:, tsl],
                                         start=(k == 0), stop=(k == KC - 1))
                    nc.scalar.activation(XG[:, tsl], pg[:], AF.Identity, bias=bg)
                    SGs = wk.tile([128, 512], F32, name="SGs", tag="SGs", bufs=1)
                    nc.scalar.activation(SGs[:], pg[:], AF.Sigmoid, bias=bg)

                    # gate partial: XG *= sigmoid(XG) ; init-scan coefficients
                    nc.gpsimd.tensor_mul(XG[:, tsl], XG[:, tsl], SGs[:])
                    nc.vector.tensor_scalar(a0[:, tsl], S0s[:], -1.0, 1.0,
                                            ALU.mult, ALU.add)
                    nc.gpsimd.tensor_mul(b0[:, tsl], S0s[:], U0s[:])

                nc.sync.dma_start(g32[:, 0:1], h0_d[b, esl, :])
                return (p, esl, XV, XD, XG, g32, a0, b0)

            def emit_S(ctx):
                p, esl, XV, XD, XG, g32, a0, b0 = ctx
                b, e = divmod(p, EC)
                g16 = wk.tile([128, T + 1], F16, name="g16", tag="g16", bufs=1)
                nc.scalar.activation(g16[:, 0:1], g32[:, 0:1], AF.Copy)
                rh_s = rh_t[:, e:e + 1]
                rd_s = rd_t[:, e:e + 1]
                omrh_s = omrh_t[:, e:e + 1]

                nc.vector.tensor_tensor_scan(
                    g16[:, 1:], a0[:], b0[:], g16[:, 0:1], ALU.mult, ALU.add)

                for it in range(NITER):
                    last = (it == NITER - 1)
                    dt = F32 if last else F16
                    tV = wk.tile([128, T], dt, name="tV", tag="tV")
                    tD = wk.tile([128, T], dt, name="tD", tag="tD")
                    gprev = g16[:, 0:T]
                    nc.vector.scalar_tensor_tensor(
                        tV[:], gprev, rh_s, XV[:], ALU.mult, ALU.add)
                    nc.vector.scalar_tensor_tensor(
                        tD[:], gprev, rd_s, XD[:], ALU.mult, ALU.add)
                    tU = wk.tile([128, T], dt, name="tU", tag="tU")
                    nc.scalar.activation(tU[:], tV[:], AF.Tanh)       # U
                    tS = wk.tile([128, T], dt, name="tS", tag="tS")
                    nc.scalar.activation(tS[:], tD[:], AF.Sigmoid)    # S
                    tU2 = wk.tile([128, T], dt, name="tU2", tag="tV")
                    nc.scalar.activation(tU2[:], tU[:], AF.Square)    # U2
                    ts_ = wk.tile([128, T], dt, name="ts_", tag="tD")
                    nc.vector.tensor_sub(ts_[:], tU[:], gprev)        # s
                    tp = wk.tile([128, T], dt, name="tp", tag="tU")
                    nc.vector.tensor_mul(tp[:], tS[:], ts_[:])        # p
                    tk = wk.tile([128, T], dt, name="tk", tag="tD")
                    nc.scalar.activation(tk[:], tU2[:], AF.Identity,
                                         bias=omrh_s, scale=rh_s)     # k
                    tA = wk.tile([128, T], dt, name="tA", tag="tV")
                    nc.vector.tensor_mul(tA[:], tS[:], tk[:])         # A'
                    tm = wk.tile([128, T], dt, name="tm", tag="tD")
                    nc.gpsimd.tensor_mul(tm[:], tA[:], gprev)         # m
                    nc.gpsimd.tensor_add(tp[:], tm[:], tp[:])         # B (over p)
                    ta = wk.tile([128, T], dt, name="ta", tag="tS")
                    nc.scalar.activation(ta[:], tA[:], AF.Identity,
                                         bias=1.0, scale=-1.0)        # a
                    gout = g32 if last else g16
                    nc.vector.tensor_tensor_scan(
                        gout[:, 1:], ta[:], tp[:], gout[:, 0:1],
                        ALU.mult, ALU.add)

                o1 = wk.tile([128, T], F32, name="o1", tag="tV")
                nc.gpsimd.tensor_mul(o1[:], g32[:, 1:], XG[:])
                nc.sync.dma_start(out_d[b, esl, :], o1[:])
                nc.sync.dma_start(h_out[b, esl, :], g32[:])

            pending = None
            for p in range(BL * EC):
                ctx = emit_G(p)
                if pending is not None:
                    emit_S(pending)
                pending = ctx
            emit_S(pending)

    _split_multi_waits(nc)
    return nc


_prog_cache = {}


def _get_program():
    if "nc" not in _prog_cache:
        _prog_cache["nc"] = _build_program()
    return _prog_cache["nc"]


def kernel(x, h0, A_log, r_delta, W_x, W_delta, W_gate, b, b_delta, b_gate,
           _profile=False):
    x = np.asarray(x, dtype=np.float32)
    h0 = np.asarray(h0, dtype=np.float32)
    A_log = np.asarray(A_log, dtype=np.float32)
    r_delta = np.asarray(r_delta, dtype=np.float32)

    nc = _get_program()

    r_h = (-np.exp(A_log)).astype(np.float32)
    rh_v = r_h.reshape(D, 1)
    rd_v = r_delta.reshape(D, 1).astype(np.float32)
    omrh_v = (1.0 - r_h).reshape(D, 1).astype(np.float32)
    bias_v = np.stack([np.asarray(v, dtype=np.float32).reshape(D, 1)
                       for v in (b, b_delta, b_gate)])  # [3, D, 1]

    wt_arrs = [_tf32_rne(np.ascontiguousarray(np.asarray(w, np.float32).T))
               for w in (W_x, W_delta, W_gate)]
    xT = _tf32_rne(np.ascontiguousarray(x.transpose(1, 2, 0)))  # [B, D, T]
    h0r = h0.reshape(B, D, 1)

    in_maps = []
    for c in range(NCORES):
        bs = slice(c * BL, (c + 1) * BL)
        m = {"xt": np.ascontiguousarray(xT[bs]),
             "bias": bias_v, "rh": rh_v, "rd": rd_v, "omrh": omrh_v,
             "h0": np.ascontiguousarray(h0r[bs])}
        for w in range(3):
            m[f"wt{w}"] = wt_arrs[w]
        in_maps.append(m)

    res = run_bass_kernel_spmd(nc, in_maps, core_ids=list(range(NCORES)),
                               trace=_profile)
    if _profile and res.exec_time_ns is not None:
        print(f"HW exec time: {res.exec_time_ns} ns")

    out_all = np.stack([res.results[c]["out_dev"] for c in range(NCORES)])
    h_all = np.stack([res.results[c]["h_dev"] for c in range(NCORES)])
    # [core, bl, d, t] -> [t, core*BL+bl, d]
    output = np.ascontiguousarray(
        out_all.transpose(3, 0, 1, 2).reshape(T, B, D))
    h = np.ascontiguousarray(
        h_all.transpose(3, 0, 1, 2).reshape(T + 1, B, D))
    return output, h
